# revision 17
# baseline (speedup 1.0000x reference)
"""FlyLoRA layer kernel for Trainium2 (8 NeuronCores, data-parallel over tokens).

Computes, for x [4, 4096, 4096], A [32, 4096], B [4096, 32], d [32], k=8:
    y = x @ A.T                      # [B, S, 32]
    mask = top-8 mask of |y + d|     # over the 32 experts
    out = (y * mask) @ B.T * 2.0     # [B, S, 4096]

Sharding: tokens (B*S = 16384) split into 8 contiguous slabs of 2048, one per
core. A/B/d are tiny and replicated. All heavy data is pre-transposed on the
host so every DMA is contiguous per partition.

Precision: mm1 (the contraction that decides top-k selection) runs in exact
fp32 on the PE; mm2 runs in fp32r (~1.8e-4 rel err on output values only).

Per core the 2048 tokens run as 2 pipelined halves of 1024 (host-contiguous
slabs, 1 MiB dense loads): half B's x-loads overlap half A's top-k and output
streaming, keeping the DMA engines continuously busy.
"""

import os

import numpy as np

import concourse.bacc as bacc
import concourse.tile as tile
from concourse import mybir
from concourse.bass_utils import run_bass_kernel_spmd
from concourse.masks import make_identity

F32 = mybir.dt.float32
F32R = mybir.dt.float32r
ALU = mybir.AluOpType

N_CORES = 8
TOKENS = 16384
TPC = 2048          # tokens per core
D = 4096            # feature dim
R = 32              # experts / lora rank
KC = D // 128       # 32 feature chunks of 128
# Asymmetric pipelined parts: the first part's output streaming covers the
# second part's load + top-k latency.
PARTS = [1280, 768]           # tokens per part (sum = TPC)
PART0 = [0, 1280]             # start token of each part
# mm1 col-packed group sizes per part (each <= 512, 4 groups max)
PGROUPS = [[512, 512, 256], [512, 256]]

_nc_cache = {}

# exposed for test.py: last BassKernelResults (for exec_time_ns when tracing)
LAST_RESULT = None


def _build_kernel():
    nc = bacc.Bacc(
        "TRN2",
        target_bir_lowering=False,
        debug=False,
        num_devices=N_CORES,
    )
    xT_d = nc.dram_tensor("xT", [D, TPC], F32, kind="ExternalInput").ap()
    atp_d = nc.dram_tensor("ATp", [128, KC * R], F32, kind="ExternalInput").ap()
    bt2_d = nc.dram_tensor("BT2", [R, D], F32, kind="ExternalInput").ap()
    drep_d = nc.dram_tensor("drep", [128, (max(PARTS) // 128) * R], F32, kind="ExternalInput").ap()
    out_d = nc.dram_tensor("out", [TPC, D], F32, kind="ExternalOutput").ap()

    with tile.TileContext(nc) as tc:
        _kernel_body(tc, out_d, xT_d, atp_d, bt2_d, drep_d)
    nc.compile()
    return nc


def _kernel_body(tc, out_d, xT_d, atp_d, bt2_d, drep_d):
    nc = tc.nc

    from contextlib import ExitStack

    with ExitStack() as ctx:
        const = ctx.enter_context(tc.tile_pool(name="const", bufs=1))
        work = ctx.enter_context(tc.tile_pool(name="work", bufs=2))
        blk = ctx.enter_context(tc.tile_pool(name="blk", bufs=2))
        xpool = ctx.enter_context(tc.tile_pool(name="xT", bufs=6))
        ypool = ctx.enter_context(tc.tile_pool(name="ypsum", bufs=2, space="PSUM"))
        tpool = ctx.enter_context(tc.tile_pool(name="tp", bufs=2, space="PSUM"))
        opool = ctx.enter_context(tc.tile_pool(name="opsum", bufs=4, space="PSUM"))
        osb = ctx.enter_context(tc.tile_pool(name="osb", bufs=4))

        # --- constants ---
        atp_sb = const.tile([128, KC * R], F32)   # [p, kc*32+r] = A[r, 128*kc+p]
        nc.sync.dma_start(out=atp_sb[:], in_=atp_d[:])
        bt2_sb = const.tile([R, D], F32)          # 2*B^T
        nc.sync.dma_start(out=bt2_sb[:], in_=bt2_d[:])
        bt2r_sb = const.tile([R, D], F32R)        # rounded for fp32r mm2
        nc.vector.tensor_copy(bt2r_sb[:], bt2_sb[:])
        drep_sb = const.tile([128, (max(PARTS) // 128) * R], F32)
        nc.sync.dma_start(out=drep_sb[:], in_=drep_d[:])
        ident = const.tile([128, 128], F32)
        make_identity(nc, ident[:])

        for h in range(len(PARTS)):
            tph = PARTS[h]
            tok0 = PART0[h]
            groups = PGROUPS[h]
            hchunks = tph // 128

            # --- phase 1: stream part's xT, mm1 (fp32) accumulates y^T; the
            # groups run as concurrent PE column-tiles in one PSUM bank ---
            ypsum = ypool.tile([128, 512], F32, tag="yps", name="yps")
            for k2 in range(KC // 2):
                xt = xpool.tile([128, 2, tph], F32, tag="xt", name="xt")
                nc.sync.dma_start(
                    out=xt[:],
                    in_=xT_d[256 * k2:256 * (k2 + 1), tok0:tok0 + tph].rearrange(
                        "(c p) t -> p c t", p=128),
                )
                for c2 in range(2):
                    kc = 2 * k2 + c2
                    lhs = atp_sb[:, R * kc:R * (kc + 1)]
                    g0 = 0
                    for g, gn in enumerate(groups):
                        nc.tensor.matmul(
                            ypsum[R * g:R * (g + 1), 0:gn],
                            lhs,
                            xt[:, c2, g0:g0 + gn],
                            start=(kc == 0),
                            stop=(kc == KC - 1),
                            tile_position=(0, R * g),
                        )
                        g0 += gn
            yT_sb = work.tile([R, tph], F32, tag="yT", name="yT")
            g0 = 0
            for g, gn in enumerate(groups):
                nc.scalar.copy(
                    yT_sb[:, g0:g0 + gn], ypsum[R * g:R * (g + 1), 0:gn]
                )
                g0 += gn

            # --- phase 1.5: transpose y^T -> token-major y [128, hchunks*32] ---
            y_sb = work.tile([128, hchunks * R], F32, tag="y", name="y")
            ytok_ps = tpool.tile([128, hchunks * R], F32, tag="tp", name="ytp")
            for c in range(hchunks):
                nc.tensor.transpose(
                    ytok_ps[:, R * c:R * (c + 1)],
                    yT_sb[:, 128 * c:128 * (c + 1)],
                    ident[0:R, 0:R],
                )
            nc.scalar.copy(y_sb[:], ytok_ps[:])

            # --- phase 2: top-8 mask of |y + d| per token ---
            zb = work.tile([128, hchunks * R], F32, tag="zb", name="zb")
            nc.vector.tensor_add(zb[:], y_sb[:], drep_sb[:, 0:hchunks * R])
            z = work.tile([128, hchunks * R], F32, tag="z", name="z")
            nc.scalar.activation(z[:], zb[:], mybir.ActivationFunctionType.Abs)
            zap = work.tile([128, hchunks * R], F32, tag="zap", name="zap")
            for c in range(hchunks):
                m8 = blk.tile([128, 8], F32, tag="m8", name="m8")
                zc = z[:, R * c:R * (c + 1)]
                nc.vector.max(out=m8[:], in_=zc)
                nc.vector.match_replace(
                    out=zap[:, R * c:R * (c + 1)],
                    in_to_replace=m8[:],
                    in_values=zc,
                    imm_value=-1.0,
                )
            mask = zb  # reuse
            nc.vector.tensor_scalar(mask[:], zap[:], 0.0, None, op0=ALU.is_lt)
            act = z  # reuse
            nc.vector.tensor_mul(act[:], y_sb[:], mask[:])

            # --- phase 3: transpose act to expert-major act^T [32, tph] (f32r) ---
            actT_sb = work.tile([R, tph], F32R, tag="actT", name="actT")
            for b in range((hchunks + 3) // 4):
                cs = list(range(4 * b, min(4 * b + 4, hchunks)))
                pt = tpool.tile([R, 512], F32, tag="tp", name="pt")
                for idx, c in enumerate(cs):
                    nc.tensor.transpose(
                        pt[:, 128 * idx:128 * (idx + 1)],
                        act[:, R * c:R * (c + 1)],
                        ident[:],
                    )
                nc.vector.tensor_copy(
                    actT_sb[:, 512 * b:512 * b + 128 * len(cs)],
                    pt[:, 0:128 * len(cs)],
                )

            # --- phase 4: mm2 (fp32r) + stores ---
            for c in range(hchunks):
                lhs = actT_sb[:, 128 * c:128 * (c + 1)]
                row0 = tok0 + 128 * c
                for half in range(2):
                    ot = osb.tile([128, D // 2], F32)
                    for n2 in range(4):
                        n = 4 * half + n2
                        ps = opool.tile([128, 512], F32)
                        nc.tensor.matmul(
                            ps[:],
                            lhs,
                            bt2r_sb[:, 512 * n:512 * (n + 1)],
                            start=True,
                            stop=True,
                        )
                        if n2 % 2 == 0:
                            nc.scalar.copy(ot[:, 512 * n2:512 * (n2 + 1)], ps[:])
                        else:
                            nc.vector.tensor_copy(ot[:, 512 * n2:512 * (n2 + 1)], ps[:])
                    nc.sync.dma_start(
                        out=out_d[row0:row0 + 128,
                                  (D // 2) * half:(D // 2) * (half + 1)],
                        in_=ot[:],
                    )

def _get_nc():
    if "nc" not in _nc_cache:
        _nc_cache["nc"] = _build_kernel()
    return _nc_cache["nc"]


def kernel(x, A, B, d, k):
    global LAST_RESULT
    assert int(k) == 8, f"kernel hardcodes k=8, got {k}"
    x = np.asarray(x, dtype=np.float32)
    A = np.asarray(A, dtype=np.float32)
    B = np.asarray(B, dtype=np.float32)
    d = np.asarray(d, dtype=np.float32)
    assert x.shape == (4, 4096, 4096) and A.shape == (R, D) and B.shape == (D, R)

    X = x.reshape(TOKENS, D)
    xT = np.ascontiguousarray(X.T)                                    # [D, TOKENS]
    ATp = np.ascontiguousarray(
        A.T.reshape(KC, 128, R).transpose(1, 0, 2).reshape(128, KC * R)
    )
    BT2 = np.ascontiguousarray(B.T) * np.float32(2.0)                 # [R, D]
    drep = np.ascontiguousarray(np.tile(d, (128, max(PARTS) // 128)))  # [128, 320]

    nc = _get_nc()
    in_maps = []
    for c in range(N_CORES):
        in_maps.append({
            "xT": np.ascontiguousarray(xT[:, c * TPC:(c + 1) * TPC]),
            "ATp": ATp,
            "BT2": BT2,
            "drep": drep,
        })
    trace = bool(int(os.environ.get("KERNEL_TRACE", "0")))
    res = run_bass_kernel_spmd(nc, in_maps, list(range(N_CORES)), trace=trace)
    LAST_RESULT = res
    outs = [res.results[c]["out"] for c in range(N_CORES)]
    full = np.concatenate(outs, axis=0)                               # [16384, 4096]
    return full.reshape(4, 4096, 4096)
